# revision 18
# baseline (speedup 1.0000x reference)
"""BertBidaf attention-flow kernel for 8 TRN2 NeuronCores (v7).

Sharding: data-parallel over batch (B=16 -> 2 batches per core); weights
replicated.

Structure (per core, 2 batches):
- qwx (mm1 moving operand) built on-chip from raw qT chunks + a packed
  const tile (saves ~1.1MB/core HBM on a DMA-bound kernel).
- PE/DVE pstate warmup: dummy ops bridge the input-DMA window so real
  matmuls run at 2.4GHz from the first tile.
- PE stream: mm1(b0), mm1(b1) tile-outer (tiles close progressively so
  softmax overlaps), then sig0/q2c0, T0, term1(b0) col-tiled, sig1,
  q2c1 (paced by cN(b1) pieces), final0, T1, term1(b1), final1.
- term1 runs as 4 concurrent col-tiled matmul chains (tile_position);
  the 4 partial [2,384] results land on partitions {0,32,64,96}+j and
  are combined by a selector matmul that shares an accumulation group
  with the t23 transposes.
- cN loads are split per c-row-tile so q2c consumes them as they land.
- Softmax never materializes a = e/den; t23 scaled by 1/den at the end.
"""

import numpy as np
import ml_dtypes

B, C, Q, D = 16, 384, 64, 2048
NCORES = 8
BPC = B // NCORES  # batches per core
NCH = D // 128     # 16 d-chunks
NW = 193           # mm1 rhs width: 64 s-cols + 1 w_c col + 2x64 P-cols
NEG = np.float32(-1e12)
BF16 = ml_dtypes.bfloat16

_cache = {}


def _build_nc():
    import concourse.bass as bass
    import concourse.bacc as bacc
    import concourse.tile as tile
    from concourse import mybir

    f32 = mybir.dt.float32
    bf16 = mybir.dt.bfloat16
    Ax = mybir.AxisListType.X
    Exp = mybir.ActivationFunctionType.Exp
    mul_op = mybir.AluOpType.mult
    add_op = mybir.AluOpType.add
    max_op = mybir.AluOpType.max

    nc = bacc.Bacc("TRN2", target_bir_lowering=False, debug=False)

    cT = nc.declare_dram_parameter("cT", [BPC, 128, NCH, C], bf16, isOutput=False)
    cN = nc.declare_dram_parameter("cN", [BPC, 128, 3, D], bf16, isOutput=False)
    qT = nc.declare_dram_parameter("qT", [BPC, 128, NCH, Q], bf16, isOutput=False)
    qxe = nc.declare_dram_parameter("qxe", [BPC, 2, NW + 384], bf16,
                                    isOutput=False)
    # cst cols: wX (64) | Wb1 (32) | W4s (32) | onc (1) | idnb (128) | sel (2)
    cst = nc.declare_dram_parameter("cst", [128, 259], bf16, isOutput=False)
    out = nc.declare_dram_parameter("out", [BPC, 2, C], f32, isOutput=True)

    with tile.TileContext(nc) as tc:
        with tc.tile_pool(name="const", bufs=1) as cp, \
             tc.tile_pool(name="io", bufs=2) as iop, \
             tc.tile_pool(name="wk", bufs=2) as wp, \
             tc.tile_pool(name="ps_s", bufs=5, space="PSUM") as ps_s_p, \
             tc.tile_pool(name="ps_q", bufs=2, space="PSUM") as ps_q_p, \
             tc.tile_pool(name="ps_t", bufs=1, space="PSUM") as ps_t_p:

            # ---- input DMAs, wire-ordered by first use ----
            # scalar ring: qT(b0), cst, qxe(b0), qT(b1), qxe(b1), cN(b1) x3
            # sync ring:   cT(b0) x4, cT(b1) x4, cN(b0) x3, out(b0), out(b1)
            cst_sb = cp.tile([128, 259], bf16, tag="cst")
            wX_sb = cst_sb[:, 0:64].rearrange("p (ch j) -> p ch j", j=4)
            Wb1_sb = cst_sb[:, 64:96].rearrange("p (ch j) -> p ch j", j=2)
            W4s_sb = cst_sb[:, 96:128].rearrange("p (ch j) -> p ch j", j=2)
            onc_sb = cst_sb[:, 128:129]
            idnb_sb = cst_sb[:, 129:257]
            sel_sb = cst_sb[:, 257:259]
            qT_sb, qwx_sb, qxe_sb = [], [], []
            for b in range(BPC):
                tq = iop.tile([128, NCH, Q], bf16, tag="qT")
                nc.scalar.dma_start(out=tq, in_=qT[b, :, :, :])
                qT_sb.append(tq)
                if b == 0:
                    nc.scalar.dma_start(out=cst_sb, in_=cst[:, :])
                tw = iop.tile([128, NCH, NW], bf16, tag="qwx")
                qwx_sb.append(tw)
                te = iop.tile([2, NW + 384], bf16, tag="qxe")
                nc.scalar.dma_start(out=te, in_=qxe[b, :, :])
                qxe_sb.append(te)

            cT_sb = [[], []]
            for b in range(BPC):
                for h in range(4):
                    th = iop.tile([128, 4, C], bf16, tag=f"cT{b}{h}")
                    nc.sync.dma_start(out=th, in_=cT[b, :, 4 * h:4 * (h + 1), :])
                    cT_sb[b].append(th)
            cN_sb = [None, None]
            tn0 = iop.tile([128, 3, D], bf16, tag="cN0")
            nc.sync.dma_start(out=tn0, in_=cN[0, :, :, :])
            cN_sb[0] = tn0
            tn1 = iop.tile([128, 3, D], bf16, tag="cN1")
            nc.scalar.dma_start(out=tn1, in_=cN[1, :, :, :])
            cN_sb[1] = tn1

            # ---- pstate warmup: dummy work so PE/DVE clocks ramp ----
            scr1 = cp.tile([128, NW], bf16, tag="scr1")
            scr2 = cp.tile([128, NW], bf16, tag="scr2")
            nc.gpsimd.memset(scr1, 0)
            ps_warm = ps_s_p.tile([128, NW], f32, tag="s", name="ps_warm")
            for w in range(38):
                nc.tensor.matmul(ps_warm, scr1[:, 0:128], scr1,
                                 start=(w == 0), stop=(w == 37))
            for w in range(20):
                nc.vector.tensor_copy(scr2, scr1)

            def cT_chunk(b, ch):
                return cT_sb[b][ch // 4][:, ch % 4, :]

            def wX_dup(j, n, c0, c1):
                # [p][ch c0:c1][i: dup n] view of wX[:, :, j]
                t = wX_sb
                return bass.AP(tensor=t.tensor, offset=t.offset + j + 4 * c0,
                               ap=[t.ap[0], [4, c1 - c0], [0, n]])

            def build(eng, b, j, lo, c0, c1):
                eng.tensor_tensor(
                    out=qwx_sb[b][:, c0:c1, lo:lo + 64],
                    in0=qT_sb[b][:, c0:c1, :], in1=wX_dup(j, 64, c0, c1),
                    op=mul_op)

            # ---- on-chip qwx build: b0 halved for an early mm1 start ----
            nc.vector.tensor_copy(qwx_sb[0][:, 0:NCH, 64:65],
                                  wX_sb[:, 0:NCH, 1:2])
            build(nc.vector, 0, 0, 0, 0, 8)
            build(nc.gpsimd, 0, 2, 65, 0, 8)
            build(nc.vector, 0, 3, 129, 0, 8)
            build(nc.vector, 0, 0, 0, 8, NCH)
            build(nc.gpsimd, 0, 2, 65, 8, NCH)
            build(nc.vector, 0, 3, 129, 8, NCH)
            build(nc.gpsimd, 1, 0, 0, 0, NCH)
            build(nc.gpsimd, 1, 2, 65, 0, NCH)
            build(nc.gpsimd, 1, 3, 129, 0, NCH)
            nc.gpsimd.tensor_copy(qwx_sb[1][:, 0:NCH, 64:65],
                                  wX_sb[:, 0:NCH, 1:2])
            # f32 identity derived on-chip (needed only at term1 time)
            idn_sb = cp.tile([128, 128], f32, tag="idn")
            nc.vector.tensor_copy(idn_sb, idnb_sb)

            # ---- mm1 both batches, tile-outer ----
            ps_s = [[None] * 3, [None] * 3]
            for b in range(BPC):
                for t in range(3):
                    ps_s[b][t] = ps_s_p.tile([128, NW], f32, tag="s",
                                             name=f"ps{b}{t}")
            for b in range(BPC):
                for t in range(3):
                    for ch in range(NCH):
                        nc.tensor.matmul(
                            ps_s[b][t],
                            cT_chunk(b, ch)[:, 128 * t:128 * (t + 1)],
                            qwx_sb[b][:, ch, :], start=(ch == 0), stop=False)
                    nc.tensor.matmul(
                        ps_s[b][t],
                        qxe_sb[b][:, NW + 128 * t:NW + 128 * (t + 1)],
                        qxe_sb[b][0:2, 0:NW], start=False, stop=True)

            # ---- softmax + terms 2/3 (ACT + DVE), per batch ----
            # t23 = (sum_i e_i * P_i) / den  (a = e/den never materialized)
            eb_t = [[], []]
            t23 = [None, None]
            for b in range(BPC):
                t23[b] = wp.tile([128, 3, 2], f32, tag="t23", name=f"t23{b}")
                for t in range(3):
                    nrm = wp.tile([128, 1], f32, tag="nrm", bufs=6)
                    nc.vector.tensor_reduce(
                        out=nrm, in_=ps_s[b][t][:, 0:64], axis=Ax, op=max_op,
                        negate=True)
                    e = wp.tile([128, 64], f32, tag="e", bufs=3)
                    den = wp.tile([128, 1], f32, tag="den", bufs=3)
                    nc.scalar.activation(e, ps_s[b][t][:, 0:64], Exp,
                                         bias=nrm, scale=1.0, accum_out=den)
                    cwc = wp.tile([128, 1], f32, tag="cwc", bufs=3)
                    nc.scalar.copy(cwc, ps_s[b][t][:, 64:65])
                    eb = wp.tile([128, 1], bf16, tag="eb", bufs=6)
                    nc.scalar.activation(eb, nrm, Exp, bias=cwc, scale=-1.0)
                    eb_t[b].append(eb)
                    rden = wp.tile([128, 1], f32, tag="rden", bufs=3)
                    nc.vector.reciprocal(rden, den)
                    e_dup = bass.AP(
                        tensor=e.tensor, offset=e.offset,
                        ap=[e.ap[0], [0, 2], e.ap[1]])
                    scr = wp.tile([128, 2, 64], f32, tag="ttscr", bufs=3)
                    nc.vector.tensor_tensor(
                        out=scr,
                        in0=ps_s[b][t][:, 65:193].rearrange(
                            "p (j i) -> p j i", j=2),
                        in1=e_dup, op=mul_op)
                    t23r = wp.tile([128, 2], f32, tag="t23r", bufs=3)
                    nc.vector.tensor_reduce(
                        out=t23r, in_=scr, axis=Ax, op=add_op)
                    nc.vector.tensor_scalar_mul(t23[b][:, t, :], t23r, rden)

            # ---- per-batch device graph pieces ----
            rb = [None, None]
            ps_q2c = [None, None]
            w14 = [None, None]

            def phase_sq(b):
                # sigma (3 tiny MMs) + q2c (12 col-tiled MMs)
                sig = ps_t_p.tile([128, C], f32, tag="t", name=f"sig{b}")
                for t in range(3):
                    nc.tensor.matmul(sig[0:1, 0:1], eb_t[b][t], onc_sb,
                                     start=(t == 0), stop=(t == 2))
                ps_q2c[b] = ps_q_p.tile([128, 512], f32, tag="q",
                                        name=f"q2c{b}")
                for t in range(3):
                    for g in range(4):
                        nc.tensor.matmul(
                            ps_q2c[b][32 * g:32 * g + 1, :],
                            eb_t[b][t],
                            cN_sb[b][:, t, 512 * g:512 * (g + 1)],
                            start=(t == 0), stop=(t == 2),
                            tile_position=(0, 32 * g))
                sigc = wp.tile([1, 1], f32, tag="sigc", name=f"sigc{b}")
                nc.vector.tensor_copy(sigc, sig[0:1, 0:1])
                rsig = wp.tile([1, 1], f32, tag="rsig", name=f"rsig{b}")
                nc.vector.reciprocal(rsig, sigc)
                rb[b] = wp.tile([128, 1], f32, tag="rb", name=f"rb{b}")
                nc.gpsimd.partition_broadcast(rb[b], rsig)

            def phase_T(b):
                # drain q2c (split DVE/ACT), transpose, w14 = W1 + W4*q2c/sig
                q2c_sb = wp.tile([128, 512], bf16, tag="q2c_sb",
                                 name=f"q2csb{b}")
                ps_T = ps_q_p.tile([128, 1024], bf16, tag="q", name=f"T{b}")
                for jh in range(4):
                    if jh % 2 == 0:
                        nc.vector.tensor_copy(
                            q2c_sb[:, 128 * jh:128 * (jh + 1)],
                            ps_q2c[b][:, 128 * jh:128 * (jh + 1)])
                    else:
                        nc.scalar.copy(
                            q2c_sb[:, 128 * jh:128 * (jh + 1)],
                            ps_q2c[b][:, 128 * jh:128 * (jh + 1)])
                    nc.tensor.transpose(ps_T[:, 128 * jh:128 * (jh + 1)],
                                        q2c_sb[:, 128 * jh:128 * (jh + 1)],
                                        idnb_sb)
                q2cT_r = wp.tile([128, NCH], bf16, tag="q2cT_r",
                                 name=f"q2cTr{b}")
                for jh in range(4):
                    src_ = ps_T[:, 128 * jh:128 * (jh + 1)]
                    v = bass.AP(
                        tensor=src_.tensor, offset=src_.offset,
                        ap=[src_.ap[0], [src_.ap[1][0] * 32, 4]])
                    nc.vector.tensor_copy(q2cT_r[:, jh::4], v)
                q2cT = wp.tile([128, NCH], bf16, tag="q2cT", name=f"q2cT{b}")
                nc.vector.tensor_scalar_mul(q2cT, q2cT_r, rb[b])
                w14[b] = wp.tile([128, NCH, 2], bf16, tag="w14",
                                 name=f"w14{b}")
                for j in range(2):
                    w4p = wp.tile([128, NCH], bf16, tag=f"w4p{j}",
                                  name=f"w4p{b}{j}")
                    nc.vector.tensor_tensor(out=w4p, in0=W4s_sb[:, :, j],
                                            in1=q2cT, op=mul_op)
                    nc.vector.tensor_tensor(out=w14[b][:, :, j], in0=w4p,
                                            in1=Wb1_sb[:, :, j], op=add_op)

            def phase_t1(b):
                if b == 0:
                    ps_t1 = ps_t_p.tile([128, C], f32, tag="t",
                                        name="t1a")[0:2, :]
                else:
                    ps_t1 = ps_q_p.tile([128, 512], f32, tag="q",
                                        name="t1b")[0:2, 0:C]
                for ch in range(NCH):
                    nc.tensor.matmul(ps_t1, w14[b][:, ch, :], cT_chunk(b, ch),
                                     start=(ch == 0), stop=False)
                for t in range(3):
                    nc.tensor.matmul(
                        ps_t1[:, 128 * t:128 * (t + 1)], t23[b][:, t, :],
                        idn_sb, is_transpose=True,
                        start=False, stop=(t == 2))
                oT = wp.tile([2, C], f32, tag="oT", name=f"oT{b}")
                nc.scalar.copy(oT, ps_t1)
                nc.sync.dma_start(out=out[b, :, :], in_=oT)

            phase_sq(0)
            phase_T(0)
            phase_t1(0)
            phase_sq(1)
            phase_T(1)
            phase_t1(1)

    nc.finalize()
    return nc


def _get_nc():
    if "nc" not in _cache:
        _cache["nc"] = _build_nc()
    return _cache["nc"]


def _prep_host(c, q, c_len, q_len, w_c, b_c, w_q, b_q, w_cq, b_cq, W_out, b_out):
    """Build per-core input maps (host-side layout/masking prep)."""
    c = np.asarray(c, np.float32)
    q = np.asarray(q, np.float32)
    c_len = np.asarray(c_len).astype(np.int64)
    q_len = np.asarray(q_len).astype(np.int64)
    w_c = np.asarray(w_c, np.float32)
    w_q = np.asarray(w_q, np.float32)
    w_cq = np.asarray(w_cq, np.float32)
    W_out = np.asarray(W_out, np.float32)
    b_out = np.asarray(b_out, np.float32)
    b_sum = float(np.asarray(b_c, np.float32) + np.asarray(b_q, np.float32)
                  + np.asarray(b_cq, np.float32))

    Mv = np.float32(BF16(-1e12))
    iq = np.arange(Q)

    Wc = W_out.reshape(4, NCH, 128, 2)  # [term, chunk, p, j]
    Wb1 = np.ascontiguousarray(Wc[0].transpose(1, 0, 2))
    W4s = np.ascontiguousarray(Wc[3].transpose(1, 0, 2))
    W2 = W_out[2048:4096]  # [D, 2]
    W3 = W_out[4096:6144]
    wX = np.empty((128, NCH, 4), np.float32)
    wX[:, :, 0] = w_cq.reshape(NCH, 128).T
    wX[:, :, 1] = w_c.reshape(NCH, 128).T
    wX[:, :, 2] = W3[:, 0].reshape(NCH, 128).T
    wX[:, :, 3] = W3[:, 1].reshape(NCH, 128).T
    cstm = np.zeros((128, 259), np.float32)
    cstm[:, 0:64] = wX.reshape(128, 64)
    cstm[:, 64:96] = Wb1.reshape(128, 32)
    cstm[:, 96:128] = W4s.reshape(128, 32)
    cstm[:, 128] = 1.0
    cstm[:, 129:257] = np.eye(128, dtype=np.float32)
    cstm[0::32, 257] = 1.0
    cstm[1::32, 258] = 1.0
    consts = dict(cst=cstm.astype(BF16))

    in_maps = []
    for core in range(NCORES):
        bs = [BPC * core + i for i in range(BPC)]
        cTm = np.empty((BPC, 128, NCH, C), BF16)
        cNm = np.empty((BPC, 128, 3, D), BF16)
        qTm = np.empty((BPC, 128, NCH, Q), BF16)
        qxm = np.zeros((BPC, 2, NW + 384), BF16)
        for i, bidx in enumerate(bs):
            cb = c[bidx]                          # [C, D]
            cTm[i] = cb.T.reshape(NCH, 128, C).transpose(1, 0, 2).astype(BF16)
            cNm[i] = cb.reshape(3, 128, D).transpose(1, 0, 2).astype(BF16)
            qb = q[bidx]                          # [Q, D]
            qTm[i] = qb.T.reshape(NCH, 128, Q).transpose(1, 0, 2).astype(BF16)
            qs = qb @ w_q + b_sum                 # [Q] f32
            low = np.where(iq >= q_len[bidx], Mv, np.float32(0))
            hi = np.where((iq < Q - 1) | (iq >= q_len[bidx]), Mv, np.float32(0))
            QW2b = qb @ W2 + b_out[None, :]       # [Q, 2] (b_out folded)
            qxm[i, 0, 0:64] = (qs + low).astype(BF16)
            qxm[i, 0, 65:129] = QW2b[:, 0].astype(BF16)
            qxm[i, 0, 129:193] = QW2b[:, 1].astype(BF16)
            qxm[i, 1, 0:64] = (hi - low).astype(BF16)
            rowind = (np.arange(C) >= c_len[bidx]).astype(np.float32)
            qxm[i, 0, NW:NW + 384] = BF16(1)
            qxm[i, 1, NW:NW + 384] = rowind.astype(BF16)
        m = dict(cT=cTm, cN=cNm, qT=qTm, qxe=qxm, **consts)
        in_maps.append(m)
    return in_maps, c_len


def kernel(**inputs):
    from concourse.bass_utils import run_bass_kernel_spmd

    nc = _get_nc()
    in_maps, c_len = _prep_host(**inputs)
    res = run_bass_kernel_spmd(nc, in_maps, core_ids=list(range(NCORES)))
    _cache["last_results"] = res

    out0 = np.empty((B, C), np.float32)
    out1 = np.empty((B, C), np.float32)
    for core in range(NCORES):
        o = res.results[core]["out"]  # [BPC, 2, C]
        for i in range(BPC):
            bidx = BPC * core + i
            out0[bidx] = o[i, 0]
            out1[bidx] = o[i, 1]
    rows = np.arange(C)[None, :]
    row_mask = (rows >= c_len[:, None]) & (rows < C - 1)
    out0 = np.where(row_mask, NEG, out0)
    out1 = np.where(row_mask, NEG, out1)
    return out0, out1


# revision 19
# speedup vs baseline: 1.0048x; 1.0048x over previous
"""BertBidaf attention-flow kernel for 8 TRN2 NeuronCores (v7).

Sharding: data-parallel over batch (B=16 -> 2 batches per core); weights
replicated.

Structure (per core, 2 batches):
- qwx (mm1 moving operand) built on-chip from raw qT chunks + a packed
  const tile (saves ~1.1MB/core HBM on a DMA-bound kernel).
- PE/DVE pstate warmup: dummy ops bridge the input-DMA window so real
  matmuls run at 2.4GHz from the first tile.
- PE stream: mm1(b0), mm1(b1) tile-outer (tiles close progressively so
  softmax overlaps), then sig0/q2c0, T0, term1(b0) col-tiled, sig1,
  q2c1 (paced by cN(b1) pieces), final0, T1, term1(b1), final1.
- term1 runs as 4 concurrent col-tiled matmul chains (tile_position);
  the 4 partial [2,384] results land on partitions {0,32,64,96}+j and
  are combined by a selector matmul that shares an accumulation group
  with the t23 transposes.
- cN loads are split per c-row-tile so q2c consumes them as they land.
- Softmax never materializes a = e/den; t23 scaled by 1/den at the end.
"""

import numpy as np
import ml_dtypes

B, C, Q, D = 16, 384, 64, 2048
NCORES = 8
BPC = B // NCORES  # batches per core
NCH = D // 128     # 16 d-chunks
NW = 193           # mm1 rhs width: 64 s-cols + 1 w_c col + 2x64 P-cols
NEG = np.float32(-1e12)
BF16 = ml_dtypes.bfloat16

_cache = {}


def _build_nc():
    import concourse.bass as bass
    import concourse.bacc as bacc
    import concourse.tile as tile
    from concourse import mybir

    f32 = mybir.dt.float32
    bf16 = mybir.dt.bfloat16
    Ax = mybir.AxisListType.X
    Exp = mybir.ActivationFunctionType.Exp
    mul_op = mybir.AluOpType.mult
    add_op = mybir.AluOpType.add
    max_op = mybir.AluOpType.max

    nc = bacc.Bacc("TRN2", target_bir_lowering=False, debug=False)

    cT = nc.declare_dram_parameter("cT", [BPC, 128, NCH, C], bf16, isOutput=False)
    cN = nc.declare_dram_parameter("cN", [BPC, 128, 3, D], bf16, isOutput=False)
    qT = nc.declare_dram_parameter("qT", [BPC, 128, NCH, Q], bf16, isOutput=False)
    qxe = nc.declare_dram_parameter("qxe", [BPC, 2, NW + 384], bf16,
                                    isOutput=False)
    # cst cols: wX (64) | Wb1 (32) | W4s (32) | onc (1) | idnb (128) | sel (2)
    cst = nc.declare_dram_parameter("cst", [128, 259], bf16, isOutput=False)
    out = nc.declare_dram_parameter("out", [BPC, 2, C], f32, isOutput=True)

    with tile.TileContext(nc) as tc:
        with tc.tile_pool(name="const", bufs=1) as cp, \
             tc.tile_pool(name="io", bufs=2) as iop, \
             tc.tile_pool(name="wk", bufs=2) as wp, \
             tc.tile_pool(name="ps_s", bufs=5, space="PSUM") as ps_s_p, \
             tc.tile_pool(name="ps_q", bufs=2, space="PSUM") as ps_q_p, \
             tc.tile_pool(name="ps_t", bufs=1, space="PSUM") as ps_t_p:

            # ---- input DMAs, wire-ordered by first use ----
            # scalar ring: qT(b0), cst, qxe(b0), qT(b1), qxe(b1), cN(b1) x3
            # sync ring:   cT(b0) x4, cT(b1) x4, cN(b0) x3, out(b0), out(b1)
            cst_sb = cp.tile([128, 259], bf16, tag="cst")
            wX_sb = cst_sb[:, 0:64].rearrange("p (ch j) -> p ch j", j=4)
            Wb1_sb = cst_sb[:, 64:96].rearrange("p (ch j) -> p ch j", j=2)
            W4s_sb = cst_sb[:, 96:128].rearrange("p (ch j) -> p ch j", j=2)
            onc_sb = cst_sb[:, 128:129]
            idnb_sb = cst_sb[:, 129:257]
            sel_sb = cst_sb[:, 257:259]
            qT_sb, qwx_sb, qxe_sb = [], [], []
            for b in range(BPC):
                tq = iop.tile([128, NCH, Q], bf16, tag="qT")
                nc.scalar.dma_start(out=tq, in_=qT[b, :, :, :])
                qT_sb.append(tq)
                if b == 0:
                    nc.scalar.dma_start(out=cst_sb, in_=cst[:, :])
                tw = iop.tile([128, NCH, NW], bf16, tag="qwx")
                qwx_sb.append(tw)
                te = iop.tile([2, NW + 384], bf16, tag="qxe")
                nc.scalar.dma_start(out=te, in_=qxe[b, :, :])
                qxe_sb.append(te)

            cT_sb = [[], []]
            for b in range(BPC):
                for h in range(4):
                    th = iop.tile([128, 4, C], bf16, tag=f"cT{b}{h}")
                    nc.sync.dma_start(out=th, in_=cT[b, :, 4 * h:4 * (h + 1), :])
                    cT_sb[b].append(th)
            # cN split per c-row tile so q2c consumes pieces as they land
            cN_sb = [None, None]
            tn0 = iop.tile([128, 3, D], bf16, tag="cN0")
            for t in range(3):
                nc.sync.dma_start(out=tn0[:, t, :], in_=cN[0, :, t, :])
            cN_sb[0] = tn0
            tn1 = iop.tile([128, 3, D], bf16, tag="cN1")
            for t in range(3):
                nc.scalar.dma_start(out=tn1[:, t, :], in_=cN[1, :, t, :])
            cN_sb[1] = tn1

            # ---- pstate warmup: dummy work so PE/DVE clocks ramp ----
            scr1 = cp.tile([128, NW], bf16, tag="scr1")
            scr2 = cp.tile([128, NW], bf16, tag="scr2")
            nc.gpsimd.memset(scr1, 0)
            for w in range(5):
                nc.gpsimd.tensor_copy(scr2, scr1)
            ps_warm = ps_s_p.tile([128, NW], f32, tag="s", name="ps_warm")
            for w in range(38):
                nc.tensor.matmul(ps_warm, scr1[:, 0:128], scr1,
                                 start=(w == 0), stop=(w == 37))
            for w in range(20):
                nc.vector.tensor_copy(scr2, scr1)

            def cT_chunk(b, ch):
                return cT_sb[b][ch // 4][:, ch % 4, :]

            def wX_dup(j, n, c0, c1):
                # [p][ch c0:c1][i: dup n] view of wX[:, :, j]
                t = wX_sb
                return bass.AP(tensor=t.tensor, offset=t.offset + j + 4 * c0,
                               ap=[t.ap[0], [4, c1 - c0], [0, n]])

            def build(eng, b, j, lo, c0, c1):
                eng.tensor_tensor(
                    out=qwx_sb[b][:, c0:c1, lo:lo + 64],
                    in0=qT_sb[b][:, c0:c1, :], in1=wX_dup(j, 64, c0, c1),
                    op=mul_op)

            # ---- on-chip qwx build: b0 halved for an early mm1 start ----
            nc.gpsimd.tensor_copy(qwx_sb[0][:, 0:NCH, 64:65],
                                  wX_sb[:, 0:NCH, 1:2])
            build(nc.vector, 0, 0, 0, 0, 8)
            build(nc.gpsimd, 0, 2, 65, 0, 8)
            build(nc.vector, 0, 3, 129, 0, 8)
            build(nc.vector, 0, 0, 0, 8, NCH)
            build(nc.gpsimd, 0, 2, 65, 8, NCH)
            build(nc.vector, 0, 3, 129, 8, NCH)
            build(nc.gpsimd, 1, 0, 0, 0, NCH)
            build(nc.gpsimd, 1, 2, 65, 0, NCH)
            build(nc.gpsimd, 1, 3, 129, 0, NCH)
            nc.gpsimd.tensor_copy(qwx_sb[1][:, 0:NCH, 64:65],
                                  wX_sb[:, 0:NCH, 1:2])
            # f32 identity derived on-chip (needed only at term1 time)
            idn_sb = cp.tile([128, 128], f32, tag="idn")
            nc.gpsimd.tensor_copy(idn_sb, idnb_sb)

            # ---- mm1 both batches, tile-outer ----
            ps_s = [[None] * 3, [None] * 3]
            for b in range(BPC):
                for t in range(3):
                    ps_s[b][t] = ps_s_p.tile([128, NW], f32, tag="s",
                                             name=f"ps{b}{t}")
            for b in range(BPC):
                for t in range(3):
                    for ch in range(NCH):
                        nc.tensor.matmul(
                            ps_s[b][t],
                            cT_chunk(b, ch)[:, 128 * t:128 * (t + 1)],
                            qwx_sb[b][:, ch, :], start=(ch == 0), stop=False)
                    nc.tensor.matmul(
                        ps_s[b][t],
                        qxe_sb[b][:, NW + 128 * t:NW + 128 * (t + 1)],
                        qxe_sb[b][0:2, 0:NW], start=False, stop=True)

            # ---- softmax + terms 2/3 (ACT + DVE), per batch ----
            # t23 = (sum_i e_i * P_i) / den  (a = e/den never materialized)
            eb_t = [[], []]
            t23 = [None, None]
            for b in range(BPC):
                t23[b] = wp.tile([128, 3, 2], f32, tag="t23", name=f"t23{b}")
                for t in range(3):
                    nrm = wp.tile([128, 1], f32, tag="nrm", bufs=6)
                    nc.vector.tensor_reduce(
                        out=nrm, in_=ps_s[b][t][:, 0:64], axis=Ax, op=max_op,
                        negate=True)
                    e = wp.tile([128, 64], f32, tag="e", bufs=3)
                    den = wp.tile([128, 1], f32, tag="den", bufs=3)
                    nc.scalar.activation(e, ps_s[b][t][:, 0:64], Exp,
                                         bias=nrm, scale=1.0, accum_out=den)
                    cwc = wp.tile([128, 1], f32, tag="cwc", bufs=3)
                    nc.scalar.copy(cwc, ps_s[b][t][:, 64:65])
                    eb = wp.tile([128, 1], bf16, tag="eb", bufs=6)
                    nc.scalar.activation(eb, nrm, Exp, bias=cwc, scale=-1.0)
                    eb_t[b].append(eb)
                    rden = wp.tile([128, 1], f32, tag="rden", bufs=3)
                    nc.vector.reciprocal(rden, den)
                    e_dup = bass.AP(
                        tensor=e.tensor, offset=e.offset,
                        ap=[e.ap[0], [0, 2], e.ap[1]])
                    scr = wp.tile([128, 2, 64], f32, tag="ttscr", bufs=3)
                    nc.vector.tensor_tensor(
                        out=scr,
                        in0=ps_s[b][t][:, 65:193].rearrange(
                            "p (j i) -> p j i", j=2),
                        in1=e_dup, op=mul_op)
                    t23r = wp.tile([128, 2], f32, tag="t23r", bufs=3)
                    nc.vector.tensor_reduce(
                        out=t23r, in_=scr, axis=Ax, op=add_op)
                    nc.vector.tensor_scalar_mul(t23[b][:, t, :], t23r, rden)

            # ---- per-batch device graph pieces ----
            rb = [None, None]
            ps_q2c = [None, None]
            w14 = [None, None]

            def phase_sq(b):
                # sigma (3 tiny MMs) + q2c (12 col-tiled MMs)
                sig = ps_t_p.tile([128, C], f32, tag="t", name=f"sig{b}")
                for t in range(3):
                    nc.tensor.matmul(sig[0:1, 0:1], eb_t[b][t], onc_sb,
                                     start=(t == 0), stop=(t == 2))
                ps_q2c[b] = ps_q_p.tile([128, 512], f32, tag="q",
                                        name=f"q2c{b}")
                for t in range(3):
                    for g in range(4):
                        nc.tensor.matmul(
                            ps_q2c[b][32 * g:32 * g + 1, :],
                            eb_t[b][t],
                            cN_sb[b][:, t, 512 * g:512 * (g + 1)],
                            start=(t == 0), stop=(t == 2),
                            tile_position=(0, 32 * g))
                sigc = wp.tile([1, 1], f32, tag="sigc", name=f"sigc{b}")
                nc.vector.tensor_copy(sigc, sig[0:1, 0:1])
                rsig = wp.tile([1, 1], f32, tag="rsig", name=f"rsig{b}")
                nc.vector.reciprocal(rsig, sigc)
                rb[b] = wp.tile([128, 1], f32, tag="rb", name=f"rb{b}")
                nc.gpsimd.partition_broadcast(rb[b], rsig)

            def phase_T(b):
                # drain q2c (split DVE/ACT), transpose, w14 = W1 + W4*q2c/sig
                q2c_sb = wp.tile([128, 512], bf16, tag="q2c_sb",
                                 name=f"q2csb{b}")
                ps_T = ps_q_p.tile([128, 1024], bf16, tag="q", name=f"T{b}")
                for jh in range(4):
                    if jh % 2 == 0:
                        nc.vector.tensor_copy(
                            q2c_sb[:, 128 * jh:128 * (jh + 1)],
                            ps_q2c[b][:, 128 * jh:128 * (jh + 1)])
                    else:
                        nc.scalar.copy(
                            q2c_sb[:, 128 * jh:128 * (jh + 1)],
                            ps_q2c[b][:, 128 * jh:128 * (jh + 1)])
                    nc.tensor.transpose(ps_T[:, 128 * jh:128 * (jh + 1)],
                                        q2c_sb[:, 128 * jh:128 * (jh + 1)],
                                        idnb_sb)
                q2cT_r = wp.tile([128, NCH], bf16, tag="q2cT_r",
                                 name=f"q2cTr{b}")
                for jh in range(4):
                    src_ = ps_T[:, 128 * jh:128 * (jh + 1)]
                    v = bass.AP(
                        tensor=src_.tensor, offset=src_.offset,
                        ap=[src_.ap[0], [src_.ap[1][0] * 32, 4]])
                    nc.vector.tensor_copy(q2cT_r[:, jh::4], v)
                q2cT = wp.tile([128, NCH], bf16, tag="q2cT", name=f"q2cT{b}")
                nc.vector.tensor_scalar_mul(q2cT, q2cT_r, rb[b])
                w14[b] = wp.tile([128, NCH, 2], bf16, tag="w14",
                                 name=f"w14{b}")
                for j in range(2):
                    w4p = wp.tile([128, NCH], bf16, tag=f"w4p{j}",
                                  name=f"w4p{b}{j}")
                    nc.vector.tensor_tensor(out=w4p, in0=W4s_sb[:, :, j],
                                            in1=q2cT, op=mul_op)
                    nc.vector.tensor_tensor(out=w14[b][:, :, j], in0=w4p,
                                            in1=Wb1_sb[:, :, j], op=add_op)

            def phase_t1(b):
                if b == 0:
                    ps_t1 = ps_t_p.tile([128, C], f32, tag="t",
                                        name="t1a")[0:2, :]
                else:
                    ps_t1 = ps_q_p.tile([128, 512], f32, tag="q",
                                        name="t1b")[0:2, 0:C]
                for ch in range(NCH):
                    nc.tensor.matmul(ps_t1, w14[b][:, ch, :], cT_chunk(b, ch),
                                     start=(ch == 0), stop=False)
                for t in range(3):
                    nc.tensor.matmul(
                        ps_t1[:, 128 * t:128 * (t + 1)], t23[b][:, t, :],
                        idn_sb, is_transpose=True,
                        start=False, stop=(t == 2))
                oT = wp.tile([2, C], f32, tag="oT", name=f"oT{b}")
                nc.scalar.copy(oT, ps_t1)
                nc.sync.dma_start(out=out[b, :, :], in_=oT)

            phase_sq(0)
            phase_T(0)
            phase_t1(0)
            phase_sq(1)
            phase_T(1)
            phase_t1(1)


    nc.finalize()
    return nc


def _get_nc():
    if "nc" not in _cache:
        _cache["nc"] = _build_nc()
    return _cache["nc"]


def _prep_host(c, q, c_len, q_len, w_c, b_c, w_q, b_q, w_cq, b_cq, W_out, b_out):
    """Build per-core input maps (host-side layout/masking prep)."""
    c = np.asarray(c, np.float32)
    q = np.asarray(q, np.float32)
    c_len = np.asarray(c_len).astype(np.int64)
    q_len = np.asarray(q_len).astype(np.int64)
    w_c = np.asarray(w_c, np.float32)
    w_q = np.asarray(w_q, np.float32)
    w_cq = np.asarray(w_cq, np.float32)
    W_out = np.asarray(W_out, np.float32)
    b_out = np.asarray(b_out, np.float32)
    b_sum = float(np.asarray(b_c, np.float32) + np.asarray(b_q, np.float32)
                  + np.asarray(b_cq, np.float32))

    Mv = np.float32(BF16(-1e12))
    iq = np.arange(Q)

    Wc = W_out.reshape(4, NCH, 128, 2)  # [term, chunk, p, j]
    Wb1 = np.ascontiguousarray(Wc[0].transpose(1, 0, 2))
    W4s = np.ascontiguousarray(Wc[3].transpose(1, 0, 2))
    W2 = W_out[2048:4096]  # [D, 2]
    W3 = W_out[4096:6144]
    wX = np.empty((128, NCH, 4), np.float32)
    wX[:, :, 0] = w_cq.reshape(NCH, 128).T
    wX[:, :, 1] = w_c.reshape(NCH, 128).T
    wX[:, :, 2] = W3[:, 0].reshape(NCH, 128).T
    wX[:, :, 3] = W3[:, 1].reshape(NCH, 128).T
    cstm = np.zeros((128, 259), np.float32)
    cstm[:, 0:64] = wX.reshape(128, 64)
    cstm[:, 64:96] = Wb1.reshape(128, 32)
    cstm[:, 96:128] = W4s.reshape(128, 32)
    cstm[:, 128] = 1.0
    cstm[:, 129:257] = np.eye(128, dtype=np.float32)
    cstm[0::32, 257] = 1.0
    cstm[1::32, 258] = 1.0
    consts = dict(cst=cstm.astype(BF16))

    in_maps = []
    for core in range(NCORES):
        bs = [BPC * core + i for i in range(BPC)]
        cTm = np.empty((BPC, 128, NCH, C), BF16)
        cNm = np.empty((BPC, 128, 3, D), BF16)
        qTm = np.empty((BPC, 128, NCH, Q), BF16)
        qxm = np.zeros((BPC, 2, NW + 384), BF16)
        for i, bidx in enumerate(bs):
            cb = c[bidx]                          # [C, D]
            cTm[i] = cb.T.reshape(NCH, 128, C).transpose(1, 0, 2).astype(BF16)
            cNm[i] = cb.reshape(3, 128, D).transpose(1, 0, 2).astype(BF16)
            qb = q[bidx]                          # [Q, D]
            qTm[i] = qb.T.reshape(NCH, 128, Q).transpose(1, 0, 2).astype(BF16)
            qs = qb @ w_q + b_sum                 # [Q] f32
            low = np.where(iq >= q_len[bidx], Mv, np.float32(0))
            hi = np.where((iq < Q - 1) | (iq >= q_len[bidx]), Mv, np.float32(0))
            QW2b = qb @ W2 + b_out[None, :]       # [Q, 2] (b_out folded)
            qxm[i, 0, 0:64] = (qs + low).astype(BF16)
            qxm[i, 0, 65:129] = QW2b[:, 0].astype(BF16)
            qxm[i, 0, 129:193] = QW2b[:, 1].astype(BF16)
            qxm[i, 1, 0:64] = (hi - low).astype(BF16)
            rowind = (np.arange(C) >= c_len[bidx]).astype(np.float32)
            qxm[i, 0, NW:NW + 384] = BF16(1)
            qxm[i, 1, NW:NW + 384] = rowind.astype(BF16)
        m = dict(cT=cTm, cN=cNm, qT=qTm, qxe=qxm, **consts)
        in_maps.append(m)
    return in_maps, c_len


def kernel(**inputs):
    from concourse.bass_utils import run_bass_kernel_spmd

    nc = _get_nc()
    in_maps, c_len = _prep_host(**inputs)
    res = run_bass_kernel_spmd(nc, in_maps, core_ids=list(range(NCORES)))
    _cache["last_results"] = res

    out0 = np.empty((B, C), np.float32)
    out1 = np.empty((B, C), np.float32)
    for core in range(NCORES):
        o = res.results[core]["out"]  # [BPC, 2, C]
        for i in range(BPC):
            bidx = BPC * core + i
            out0[bidx] = o[i, 0]
            out1[bidx] = o[i, 1]
    rows = np.arange(C)[None, :]
    row_mask = (rows >= c_len[:, None]) & (rows < C - 1)
    out0 = np.where(row_mask, NEG, out0)
    out1 = np.where(row_mask, NEG, out1)
    return out0, out1
